# revision 1
# baseline (speedup 1.0000x reference)
"""Trainium2 kernel for nn_LinearAutoDecoder (cluster-routed per-row 3x95 matvec).

out[i] = W[3*c_i : 3*c_i+3] @ x_i  with W = [W_pos | W_feat] in R^{384x95}.

Strategy: rows are grouped by cluster (each cluster's rows sharded round-robin
across the 8 cores so every core runs the identical static schedule), X is
streamed in a pre-transposed [96, R] layout, and the device does dense fp32r
matmuls (full PE rate at moving dim 512) with the per-cluster [96, 3]
stationary baked into the instruction stream as AP offsets. The host scatters
the [3, R] result back to original row order.
"""

import os
import sys

for _p in (
    "/root/.axon_site",
    "/root/.axon_site/_ro/trn_rl_repo",
    "/root/.axon_site/_ro/pypackages",
    "/opt/trn_rl_repo",
    "/opt/pypackages",
):
    if os.path.isdir(_p) and _p not in sys.path:
        sys.path.append(_p)

import numpy as np

N_CORES = 8
F = 95          # feature dim (63 pos + 32 latent)
FP = 96         # padded feature dim (matmul K)
NCL = 128       # clusters
ST = 512        # rows per supertile (matmul moving dim)
CH = 16         # supertiles per DMA chunk
USE_FP32R = True

_prog_cache = {}


def _build_program(schedule, R):
    from contextlib import ExitStack

    import concourse.bacc as bacc
    import concourse.tile as tile
    import concourse.tile_sem_assignment as tsa
    from concourse import mybir

    # Keep the end-of-kernel drain wait fan-in within walrus' per-instruction
    # sync-wait budget: two SWDGE completion lanes instead of eight.
    tsa.NUM_SWDGE_GLOBAL_SEMS = 2

    nc = bacc.Bacc(
        "TRN2", target_bir_lowering=False, debug=False, num_devices=N_CORES
    )
    xt = nc.dram_tensor("xt", [FP, R], mybir.dt.float32, kind="ExternalInput").ap()
    wt = nc.dram_tensor(
        "wt", [FP, 3 * NCL], mybir.dt.float32, kind="ExternalInput"
    ).ap()
    ot = nc.dram_tensor("ot", [3, R], mybir.dt.float32, kind="ExternalOutput").ap()

    T = len(schedule)
    assert T % CH == 0 and T * ST == R
    r_dt = mybir.dt.float32r if USE_FP32R else mybir.dt.float32

    with tile.TileContext(nc, trace_sim=False) as tc, ExitStack() as ctx:
        wpool = ctx.enter_context(tc.tile_pool(name="w", bufs=1))
        xpool = ctx.enter_context(tc.tile_pool(name="x", bufs=2))
        opool = ctx.enter_context(tc.tile_pool(name="o", bufs=2))
        ppool = ctx.enter_context(tc.tile_pool(name="p", bufs=4, space="PSUM"))

        w_sb = wpool.tile([FP, 3 * NCL], r_dt)
        nc.gpsimd.dma_start(w_sb[:], wt[:])

        for ch in range(T // CH):
            x_sb = xpool.tile([FP, CH * ST], r_dt)
            nc.gpsimd.dma_start(
                x_sb[:], xt[:, ch * CH * ST : (ch + 1) * CH * ST]
            )
            o_sb = opool.tile([3, CH * ST], mybir.dt.float32)
            for jp in range(CH // 2):
                ps = ppool.tile([3, 2 * ST], mybir.dt.float32)
                for h in range(2):
                    j = 2 * jp + h
                    c = schedule[ch * CH + j]
                    nc.tensor.matmul(
                        ps[:, h * ST : (h + 1) * ST],
                        lhsT=w_sb[:, 3 * c : 3 * c + 3],
                        rhs=x_sb[:, j * ST : (j + 1) * ST],
                        start=True,
                        stop=True,
                    )
                sl = slice(2 * jp * ST, (2 * jp + 2) * ST)
                if jp % 2 == 0:
                    nc.vector.tensor_copy(o_sb[:, sl], ps[:])
                else:
                    nc.scalar.copy(o_sb[:, sl], ps[:])
            nc.gpsimd.dma_start(
                ot[:, ch * CH * ST : (ch + 1) * CH * ST], o_sb[:]
            )
    nc.compile()
    return nc


def kernel(X, cluster_ids, W_pos, W_feat):
    X = np.asarray(X, dtype=np.float32)
    ids = np.asarray(cluster_ids, dtype=np.int32)
    W_pos = np.asarray(W_pos, dtype=np.float32)
    W_feat = np.asarray(W_feat, dtype=np.float32)
    N = X.shape[0]

    W = np.concatenate([W_pos, W_feat], axis=1)  # [384, 95]
    WT = np.zeros((FP, 3 * NCL), dtype=np.float32)
    WT[:F, :] = W.T  # column 3c+j = W[3c+j, :] (zero-padded K row 95)

    order = np.argsort(ids, kind="stable")
    counts = np.bincount(ids, minlength=NCL)
    offs = np.concatenate([[0], np.cumsum(counts)])
    Ks = [
        int(-(-(-(-int(counts[c]) // N_CORES)) // ST)) if counts[c] else 0
        for c in range(NCL)
    ]
    # Ks[c] = ceil(ceil(n_c / 8) / 512)
    Ks = [
        ((int(counts[c]) + N_CORES - 1) // N_CORES + ST - 1) // ST
        for c in range(NCL)
    ]
    schedule = [c for c in range(NCL) for _ in range(Ks[c])]
    while len(schedule) % CH:
        schedule.append(0)
    T = len(schedule)
    R = T * ST

    # Per-core row lists: cluster c's shard for core m is Ic[m::8], padded to
    # Ks[c]*512 with index N (an all-zero row appended to X).
    rows = np.full((N_CORES, R), N, dtype=np.int64)
    tile_base = 0
    for c in range(NCL):
        Ic = order[offs[c] : offs[c + 1]]
        for m in range(N_CORES):
            sh = Ic[m::N_CORES]
            rows[m, tile_base * ST : tile_base * ST + len(sh)] = sh
        tile_base += Ks[c]

    Xaug = np.zeros((N + 1, FP), dtype=np.float32)
    Xaug[:N, :F] = X

    in_maps = []
    for m in range(N_CORES):
        Xt = np.ascontiguousarray(Xaug[rows[m]].T)  # [96, R]
        in_maps.append({"xt": Xt, "wt": WT})

    key = (tuple(schedule), R)
    if key not in _prog_cache:
        _prog_cache.clear()
        _prog_cache[key] = _build_program(schedule, R)
    nc = _prog_cache[key]

    from concourse.bass_utils import run_bass_kernel_spmd

    res = run_bass_kernel_spmd(nc, in_maps, list(range(N_CORES)))

    out = np.zeros((N, 3), dtype=np.float32)
    for m in range(N_CORES):
        otm = res.results[m]["ot"]  # [3, R]
        valid = rows[m] != N
        out[rows[m][valid]] = otm.T[valid]
    return out



# revision 8
# speedup vs baseline: 2.4655x; 2.4655x over previous
"""Trainium2 kernel for nn_LinearAutoDecoder (cluster-routed per-row 3x95 matvec).

out[i] = W[3*c_i : 3*c_i+3] @ x_i  with W = [W_pos | W_feat] in R^{384x95}.

Strategy: rows are grouped by cluster (each cluster's rows sharded round-robin
across the 8 cores so every core runs the identical static schedule), X is
streamed in a pre-transposed [95, R] bf16 layout, and the device does dense
bf16 matmuls (512-row moving tiles) with the per-cluster [95, 3] stationary
baked into the instruction stream as AP offsets. Outputs of 4 consecutive
supertiles are packed into one PSUM bank at partition offsets 0/32/64/96 via
explicit PE column tile positions, drained with one cheap 512-wide copy per
bank, and streamed out bf16 through a strided 12-lane DMA. X loads ride the
SP HWDGE queue while output stores ride the Pool SWDGE queue so stores never
head-of-line-block the X stream. The host scatters the result back to
original row order.
"""

import os
import sys

for _p in (
    "/root/.axon_site",
    "/root/.axon_site/_ro/trn_rl_repo",
    "/root/.axon_site/_ro/pypackages",
    "/opt/trn_rl_repo",
    "/opt/pypackages",
):
    if os.path.isdir(_p) and _p not in sys.path:
        sys.path.append(_p)

import numpy as np
import ml_dtypes

N_CORES = 8
F = 95          # feature dim (63 pos + 32 latent) = matmul K
NCL = 128       # clusters
ST = 512        # rows per supertile (matmul moving dim, one PSUM bank wide)
BANK = 4        # supertiles packed per PSUM bank (partition offsets 0/32/64/96)
OUTG = 16       # banks per output tile (drained by 4 per-quadrant DMAs)
SUB = 16        # supertiles per X-load DMA
WARMUP = 10     # PE pstate warmup matmuls

_prog_cache = {}


def _build_program(schedule, R):
    from contextlib import ExitStack

    import concourse.bacc as bacc
    import concourse.tile as tile
    import concourse.tile_sem_assignment as tsa
    from concourse import mybir

    # Keep the end-of-kernel drain wait fan-in within walrus' per-instruction
    # sync-wait budget: two SWDGE completion lanes instead of eight.
    tsa.NUM_SWDGE_GLOBAL_SEMS = 2

    nc = bacc.Bacc(
        "TRN2", target_bir_lowering=False, debug=False, num_devices=N_CORES
    )
    bf16 = mybir.dt.bfloat16
    T = len(schedule)
    assert T % (BANK * OUTG) == 0 and T % SUB == 0 and T * ST == R
    NB = T // BANK

    xt = nc.dram_tensor("xt", [F, R], bf16, kind="ExternalInput").ap()
    wt = nc.dram_tensor("wt", [F, 3 * NCL], bf16, kind="ExternalInput").ap()
    ot = nc.dram_tensor("ot", [3 * BANK, NB * ST], bf16, kind="ExternalOutput").ap()

    with tile.TileContext(nc, trace_sim=False) as tc, ExitStack() as ctx:
        wpool = ctx.enter_context(tc.tile_pool(name="w", bufs=1))
        xpool = ctx.enter_context(tc.tile_pool(name="x", bufs=6))
        opool = ctx.enter_context(tc.tile_pool(name="o", bufs=3))
        ppool = ctx.enter_context(tc.tile_pool(name="p", bufs=6, space="PSUM"))
        wpps = ctx.enter_context(tc.tile_pool(name="wp", bufs=1, space="PSUM"))

        w_sb = wpool.tile([F, 3 * NCL], bf16)
        nc.scalar.dma_start(w_sb[:], wt[:])

        # PE pstate warmup: ~10 x 384-row matmuls (~3.5us) hidden under the
        # first X DMA so real matmuls start at the 2.4 GHz pstate.
        wps = wpps.tile([3, 3 * NCL], mybir.dt.float32)
        for _ in range(WARMUP):
            nc.tensor.matmul(
                wps[:], lhsT=w_sb[:, 0:3], rhs=w_sb[:], start=True, stop=True
            )

        x_sb = {}
        o_sb = None
        for b in range(NB):
            t0 = b * BANK  # first supertile of this bank
            if t0 % SUB == 0:
                xs = xpool.tile([F, SUB * ST], bf16)
                nc.sync.dma_start(
                    xs[:], xt[:, t0 * ST : (t0 + SUB) * ST]
                )
                x_sb = {"tile": xs, "base": t0}
            if b % OUTG == 0:
                o_sb = opool.tile([128, OUTG * ST], bf16)
            ps = ppool.tile([128, ST], mybir.dt.float32)
            for q in range(BANK):
                t = t0 + q
                c = schedule[t]
                off = t - x_sb["base"]
                nc.tensor.matmul(
                    ps[32 * q : 32 * q + 3, :],
                    lhsT=w_sb[:, 3 * c : 3 * c + 3],
                    rhs=x_sb["tile"][:, off * ST : (off + 1) * ST],
                    start=True,
                    stop=True,
                    tile_position=(0, 32 * q),
                )
            u = b % OUTG
            if b % 2 == 0:
                nc.vector.tensor_copy(o_sb[:, u * ST : (u + 1) * ST], ps[:])
            else:
                nc.scalar.copy(o_sb[:, u * ST : (u + 1) * ST], ps[:])
            if u == OUTG - 1:
                # Drain the 12 valid lanes: one plain 2D DMA per quadrant.
                k = b // OUTG
                for q in range(BANK):
                    nc.gpsimd.dma_start(
                        ot[3 * q : 3 * q + 3, k * OUTG * ST : (k + 1) * OUTG * ST],
                        o_sb[32 * q : 32 * q + 3, :],
                    )
    nc.compile()
    return nc


def kernel(X, cluster_ids, W_pos, W_feat):
    X = np.asarray(X, dtype=np.float32)
    ids = np.asarray(cluster_ids, dtype=np.int32)
    W_pos = np.asarray(W_pos, dtype=np.float32)
    W_feat = np.asarray(W_feat, dtype=np.float32)
    N = X.shape[0]

    W = np.concatenate([W_pos, W_feat], axis=1)  # [384, 95]
    WT = np.ascontiguousarray(W.T).astype(ml_dtypes.bfloat16)  # [95, 384]

    order = np.argsort(ids, kind="stable")
    counts = np.bincount(ids, minlength=NCL)
    # Ks[c] = ceil(ceil(n_c / 8) / 512) supertiles per cluster per core
    Ks = [
        ((int(counts[c]) + N_CORES - 1) // N_CORES + ST - 1) // ST
        for c in range(NCL)
    ]
    schedule = [c for c in range(NCL) for _ in range(Ks[c])]
    while len(schedule) % (BANK * OUTG):
        schedule.append(0)
    T = len(schedule)
    R = T * ST
    NB = T // BANK

    offs = np.concatenate([[0], np.cumsum(counts)])
    # Per-core row lists: cluster c's shard for core m is Ic[m::8], padded to
    # Ks[c]*512 with index N (an all-zero row appended to X).
    rows = np.full((N_CORES, R), N, dtype=np.int64)
    tile_base = 0
    for c in range(NCL):
        Ic = order[offs[c] : offs[c + 1]]
        for m in range(N_CORES):
            sh = Ic[m::N_CORES]
            rows[m, tile_base * ST : tile_base * ST + len(sh)] = sh
        tile_base += Ks[c]

    Xaug = np.zeros((N + 1, F), dtype=ml_dtypes.bfloat16)
    Xaug[:N] = X

    in_maps = []
    for m in range(N_CORES):
        Xt = np.ascontiguousarray(Xaug[rows[m]].T)  # [95, R] bf16
        in_maps.append({"xt": Xt, "wt": WT})

    key = (tuple(schedule), R)
    if key not in _prog_cache:
        _prog_cache.clear()
        _prog_cache[key] = _build_program(schedule, R)
    nc = _prog_cache[key]

    from concourse.bass_utils import run_bass_kernel_spmd

    res = run_bass_kernel_spmd(nc, in_maps, list(range(N_CORES)))

    out = np.zeros((N, 3), dtype=np.float32)
    for m in range(N_CORES):
        otm = np.asarray(res.results[m]["ot"])  # [12, NB*512] bf16
        # ot[3q+r, 512b+m] = out channel r of supertile (b*BANK+q) row m
        full = (
            otm.reshape(BANK, 3, NB, ST)
            .transpose(2, 0, 3, 1)
            .reshape(R, 3)
            .astype(np.float32)
        )
        valid = rows[m] != N
        out[rows[m][valid]] = full[valid]
    return out


# revision 9
# speedup vs baseline: 2.6006x; 1.0548x over previous
"""Trainium2 kernel for nn_LinearAutoDecoder (cluster-routed per-row 3x95 matvec).

out[i] = W[3*c_i : 3*c_i+3] @ x_i  with W = [W_pos | W_feat] in R^{384x95}.

Strategy: rows are grouped by cluster (each cluster's rows sharded round-robin
across the 8 cores so every core runs the identical static schedule), X is
streamed in a pre-transposed [95, R] bf16 layout, and the device does dense
bf16 matmuls (512-row moving tiles) with the per-cluster [95, 3] stationary
baked into the instruction stream as AP offsets. Outputs of 4 consecutive
supertiles are packed into one PSUM bank at partition offsets 0/32/64/96 via
explicit PE column tile positions, drained with one cheap 512-wide copy per
bank, and streamed out bf16 through a strided 12-lane DMA. X loads ride the
SP HWDGE queue while output stores ride the Pool SWDGE queue so stores never
head-of-line-block the X stream. The host scatters the result back to
original row order.
"""

import os
import sys

for _p in (
    "/root/.axon_site",
    "/root/.axon_site/_ro/trn_rl_repo",
    "/root/.axon_site/_ro/pypackages",
    "/opt/trn_rl_repo",
    "/opt/pypackages",
):
    if os.path.isdir(_p) and _p not in sys.path:
        sys.path.append(_p)

import numpy as np
import ml_dtypes

N_CORES = 8
F = 95          # feature dim (63 pos + 32 latent) = matmul K
NCL = 128       # clusters
ST = 512        # rows per supertile (matmul moving dim, one PSUM bank wide)
BANK = 4        # supertiles packed per PSUM bank (partition offsets 0/32/64/96)
OUTG = 16       # banks per output tile (drained by 4 per-quadrant DMAs)
SUB = 16        # supertiles per X-load DMA
WARMUP = 10     # PE pstate warmup matmuls

_prog_cache = {}


def _build_program(schedule, R):
    from contextlib import ExitStack

    import concourse.bacc as bacc
    import concourse.tile as tile
    import concourse.tile_sem_assignment as tsa
    from concourse import mybir

    # Keep the end-of-kernel drain wait fan-in within walrus' per-instruction
    # sync-wait budget: two SWDGE completion lanes instead of eight.
    tsa.NUM_SWDGE_GLOBAL_SEMS = 2

    nc = bacc.Bacc(
        "TRN2", target_bir_lowering=False, debug=False, num_devices=N_CORES
    )
    bf16 = mybir.dt.bfloat16
    T = len(schedule)
    assert T % (BANK * OUTG) == 0 and T % SUB == 0 and T * ST == R
    NB = T // BANK

    xt = nc.dram_tensor("xt", [F, R], bf16, kind="ExternalInput").ap()
    wt = nc.dram_tensor("wt", [F, 3 * NCL], bf16, kind="ExternalInput").ap()
    ot = nc.dram_tensor("ot", [3 * BANK, NB * ST], bf16, kind="ExternalOutput").ap()

    with tile.TileContext(nc, trace_sim=False) as tc, ExitStack() as ctx:
        wpool = ctx.enter_context(tc.tile_pool(name="w", bufs=1))
        xpool = ctx.enter_context(tc.tile_pool(name="x", bufs=6))
        opool = ctx.enter_context(tc.tile_pool(name="o", bufs=3))
        ppool = ctx.enter_context(tc.tile_pool(name="p", bufs=6, space="PSUM"))
        wpps = ctx.enter_context(tc.tile_pool(name="wp", bufs=1, space="PSUM"))

        w_sb = wpool.tile([F, 3 * NCL], bf16)
        nc.scalar.dma_start(w_sb[:], wt[:])

        # PE pstate warmup: ~10 x 384-row matmuls (~3.5us) hidden under the
        # first X DMA so real matmuls start at the 2.4 GHz pstate.
        wps = wpps.tile([3, 3 * NCL], mybir.dt.float32)
        for _ in range(WARMUP):
            nc.tensor.matmul(
                wps[:], lhsT=w_sb[:, 0:3], rhs=w_sb[:], start=True, stop=True
            )

        # X-load chunk boundaries: SUB supertiles each, the last chunk split
        # into 4 small pieces so the tail matmuls start as soon as possible.
        xchunks = []
        for t0 in range(0, T - SUB, SUB):
            xchunks.append((t0, SUB))
        for t0 in range(T - SUB, T, SUB // 4):
            xchunks.append((t0, SUB // 4))

        x_sb = {}
        o_sb = None
        ci = 0
        for b in range(NB):
            t0 = b * BANK  # first supertile of this bank
            while ci < len(xchunks) and xchunks[ci][0] <= t0:
                c0, clen = xchunks[ci]
                xs = xpool.tile([F, clen * ST], bf16)
                nc.sync.dma_start(xs[:], xt[:, c0 * ST : (c0 + clen) * ST])
                x_sb[c0] = xs
                ci += 1
            if b % OUTG == 0:
                o_sb = opool.tile([128, OUTG * ST], bf16)
            ps = ppool.tile([128, ST], mybir.dt.float32)
            for q in range(BANK):
                t = t0 + q
                c = schedule[t]
                base = max(c0_ for c0_ in x_sb if c0_ <= t)
                off = t - base
                nc.tensor.matmul(
                    ps[32 * q : 32 * q + 3, :],
                    lhsT=w_sb[:, 3 * c : 3 * c + 3],
                    rhs=x_sb[base][:, off * ST : (off + 1) * ST],
                    start=True,
                    stop=True,
                    tile_position=(0, 32 * q),
                )
            u = b % OUTG
            nc.vector.tensor_copy(o_sb[:, u * ST : (u + 1) * ST], ps[:])
            if (b + 1) % (SUB // BANK) == 0 and b != NB - 1:
                # Keep the PE pstate hot across the idle at each X-chunk
                # boundary with a few tiny dummy matmuls.
                for _ in range(4):
                    nc.tensor.matmul(
                        wps[:, 0:256],
                        lhsT=w_sb[:, 0:3],
                        rhs=w_sb[:, 0:256],
                        start=True,
                        stop=True,
                    )
            if u == OUTG - 1:
                # Drain the 12 valid lanes: one plain 2D DMA per quadrant,
                # alternating queues; the final group rides the fast HWDGE.
                k = b // OUTG
                eng = nc.gpsimd if k % 2 == 0 else nc.scalar
                for q in range(BANK):
                    eng.dma_start(
                        ot[3 * q : 3 * q + 3, k * OUTG * ST : (k + 1) * OUTG * ST],
                        o_sb[32 * q : 32 * q + 3, :],
                    )
    nc.compile()
    return nc


def kernel(X, cluster_ids, W_pos, W_feat):
    X = np.asarray(X, dtype=np.float32)
    ids = np.asarray(cluster_ids, dtype=np.int32)
    W_pos = np.asarray(W_pos, dtype=np.float32)
    W_feat = np.asarray(W_feat, dtype=np.float32)
    N = X.shape[0]

    W = np.concatenate([W_pos, W_feat], axis=1)  # [384, 95]
    WT = np.ascontiguousarray(W.T).astype(ml_dtypes.bfloat16)  # [95, 384]

    order = np.argsort(ids, kind="stable")
    counts = np.bincount(ids, minlength=NCL)
    # Ks[c] = ceil(ceil(n_c / 8) / 512) supertiles per cluster per core
    Ks = [
        ((int(counts[c]) + N_CORES - 1) // N_CORES + ST - 1) // ST
        for c in range(NCL)
    ]
    schedule = [c for c in range(NCL) for _ in range(Ks[c])]
    while len(schedule) % (BANK * OUTG):
        schedule.append(0)
    T = len(schedule)
    R = T * ST
    NB = T // BANK

    offs = np.concatenate([[0], np.cumsum(counts)])
    # Per-core row lists: cluster c's shard for core m is Ic[m::8], padded to
    # Ks[c]*512 with index N (an all-zero row appended to X).
    rows = np.full((N_CORES, R), N, dtype=np.int64)
    tile_base = 0
    for c in range(NCL):
        Ic = order[offs[c] : offs[c + 1]]
        for m in range(N_CORES):
            sh = Ic[m::N_CORES]
            rows[m, tile_base * ST : tile_base * ST + len(sh)] = sh
        tile_base += Ks[c]

    Xaug = np.zeros((N + 1, F), dtype=ml_dtypes.bfloat16)
    Xaug[:N] = X

    in_maps = []
    for m in range(N_CORES):
        Xt = np.ascontiguousarray(Xaug[rows[m]].T)  # [95, R] bf16
        in_maps.append({"xt": Xt, "wt": WT})

    key = (tuple(schedule), R)
    if key not in _prog_cache:
        _prog_cache.clear()
        _prog_cache[key] = _build_program(schedule, R)
    nc = _prog_cache[key]

    from concourse.bass_utils import run_bass_kernel_spmd

    res = run_bass_kernel_spmd(nc, in_maps, list(range(N_CORES)))

    out = np.zeros((N, 3), dtype=np.float32)
    for m in range(N_CORES):
        otm = np.asarray(res.results[m]["ot"])  # [12, NB*512] bf16
        # ot[3q+r, 512b+m] = out channel r of supertile (b*BANK+q) row m
        full = (
            otm.reshape(BANK, 3, NB, ST)
            .transpose(2, 0, 3, 1)
            .reshape(R, 3)
            .astype(np.float32)
        )
        valid = rows[m] != N
        out[rows[m][valid]] = full[valid]
    return out


# revision 11
# speedup vs baseline: 2.6007x; 1.0000x over previous
"""Trainium2 kernel for nn_LinearAutoDecoder (cluster-routed per-row 3x95 matvec).

out[i] = W[3*c_i : 3*c_i+3] @ x_i  with W = [W_pos | W_feat] in R^{384x95}.

Strategy: rows are grouped by cluster (each cluster's rows sharded round-robin
across the 8 cores so every core runs the identical static schedule), X is
streamed in a pre-transposed [95, R] bf16 layout, and the device does dense
bf16 matmuls (512-row moving tiles) with the per-cluster [95, 3] stationary
baked into the instruction stream as AP offsets. Outputs of 4 consecutive
supertiles are packed into one PSUM bank at partition offsets 0/32/64/96 via
explicit PE column tile positions, drained with one cheap 512-wide copy per
bank, and streamed out bf16 through a strided 12-lane DMA. X loads ride the
SP HWDGE queue while output stores ride the Pool SWDGE queue so stores never
head-of-line-block the X stream. The host scatters the result back to
original row order.
"""

import os
import sys

for _p in (
    "/root/.axon_site",
    "/root/.axon_site/_ro/trn_rl_repo",
    "/root/.axon_site/_ro/pypackages",
    "/opt/trn_rl_repo",
    "/opt/pypackages",
):
    if os.path.isdir(_p) and _p not in sys.path:
        sys.path.append(_p)

import numpy as np
import ml_dtypes

N_CORES = 8
F = 95          # feature dim (63 pos + 32 latent) = matmul K
NCL = 128       # clusters
ST = 512        # rows per supertile (matmul moving dim, one PSUM bank wide)
BANK = 4        # supertiles packed per PSUM bank (partition offsets 0/32/64/96)
OUTG = 16       # banks per output tile (drained by 4 per-quadrant DMAs)
SUB = 16        # supertiles per X-load DMA
WARMUP = 10     # PE pstate warmup matmuls

_prog_cache = {}


def _build_program(schedule, R):
    from contextlib import ExitStack

    import concourse.bacc as bacc
    import concourse.tile as tile
    import concourse.tile_sem_assignment as tsa
    from concourse import mybir

    # Keep the end-of-kernel drain wait fan-in within walrus' per-instruction
    # sync-wait budget: two SWDGE completion lanes instead of eight.
    tsa.NUM_SWDGE_GLOBAL_SEMS = 2

    nc = bacc.Bacc(
        "TRN2", target_bir_lowering=False, debug=False, num_devices=N_CORES
    )
    bf16 = mybir.dt.bfloat16
    T = len(schedule)
    assert T % (BANK * OUTG) == 0 and T % SUB == 0 and T * ST == R
    NB = T // BANK

    xt = nc.dram_tensor("xt", [F, R], bf16, kind="ExternalInput").ap()
    wt = nc.dram_tensor("wt", [F, 3 * NCL], bf16, kind="ExternalInput").ap()
    ot = nc.dram_tensor("ot", [3 * BANK, NB * ST], bf16, kind="ExternalOutput").ap()

    with tile.TileContext(nc, trace_sim=False) as tc, ExitStack() as ctx:
        wpool = ctx.enter_context(tc.tile_pool(name="w", bufs=1))
        xpool = ctx.enter_context(tc.tile_pool(name="x", bufs=7))
        opool = ctx.enter_context(tc.tile_pool(name="o", bufs=3))
        ppool = ctx.enter_context(tc.tile_pool(name="p", bufs=6, space="PSUM"))
        wpps = ctx.enter_context(tc.tile_pool(name="wp", bufs=1, space="PSUM"))

        w_sb = wpool.tile([F, 3 * NCL], bf16)
        nc.scalar.dma_start(w_sb[:], wt[:])

        # PE pstate warmup: ~10 x 384-row matmuls (~3.5us) hidden under the
        # first X DMA so real matmuls start at the 2.4 GHz pstate.
        wps = wpps.tile([3, 3 * NCL], mybir.dt.float32)
        for _ in range(WARMUP):
            nc.tensor.matmul(
                wps[:], lhsT=w_sb[:, 0:3], rhs=w_sb[:], start=True, stop=True
            )

        # X-load chunk boundaries: SUB supertiles each, the last chunk split
        # into 4 small pieces so the tail matmuls start as soon as possible.
        xchunks = []
        for t0 in range(0, T - SUB, SUB):
            xchunks.append((t0, SUB))
        for t0 in range(T - SUB, T, SUB // 4):
            xchunks.append((t0, SUB // 4))

        x_sb = {}
        o_sb = None
        ci = 0
        for b in range(NB):
            t0 = b * BANK  # first supertile of this bank
            while ci < len(xchunks) and xchunks[ci][0] <= t0:
                c0, clen = xchunks[ci]
                xs = xpool.tile([F, clen * ST], bf16)
                nc.sync.dma_start(xs[:], xt[:, c0 * ST : (c0 + clen) * ST])
                x_sb[c0] = xs
                ci += 1
            if b % OUTG == 0:
                o_sb = opool.tile([128, OUTG * ST], bf16)
            ps = ppool.tile([128, ST], mybir.dt.float32)
            for q in range(BANK):
                t = t0 + q
                c = schedule[t]
                base = max(c0_ for c0_ in x_sb if c0_ <= t)
                off = t - base
                nc.tensor.matmul(
                    ps[32 * q : 32 * q + 3, :],
                    lhsT=w_sb[:, 3 * c : 3 * c + 3],
                    rhs=x_sb[base][:, off * ST : (off + 1) * ST],
                    start=True,
                    stop=True,
                    tile_position=(0, 32 * q),
                )
            u = b % OUTG
            if b % 2 == 0:
                nc.vector.tensor_copy(o_sb[:, u * ST : (u + 1) * ST], ps[:])
            else:
                nc.scalar.copy(o_sb[:, u * ST : (u + 1) * ST], ps[:])
            if (b + 1) % (SUB // BANK) == 0 and b != NB - 1:
                # Keep the PE pstate hot across the idle at each X-chunk
                # boundary with a few tiny dummy matmuls.
                for _ in range(4):
                    nc.tensor.matmul(
                        wps[:, 0:256],
                        lhsT=w_sb[:, 0:3],
                        rhs=w_sb[:, 0:256],
                        start=True,
                        stop=True,
                    )
            if u == OUTG - 1:
                # Drain the 12 valid lanes: one plain 2D DMA per quadrant.
                # Mid-stream groups ride the otherwise-idle Pool SWDGE queue
                # (queue-head waits there block nothing); the final group is
                # split across the SP and Act HWDGE queues, both free by then.
                k = b // OUTG
                last = b == NB - 1
                for q in range(BANK):
                    eng = nc.gpsimd if not last else (nc.sync if q < 2 else nc.scalar)
                    eng.dma_start(
                        ot[3 * q : 3 * q + 3, k * OUTG * ST : (k + 1) * OUTG * ST],
                        o_sb[32 * q : 32 * q + 3, :],
                    )
    nc.compile()
    return nc


def kernel(X, cluster_ids, W_pos, W_feat):
    X = np.asarray(X, dtype=np.float32)
    ids = np.asarray(cluster_ids, dtype=np.int32)
    W_pos = np.asarray(W_pos, dtype=np.float32)
    W_feat = np.asarray(W_feat, dtype=np.float32)
    N = X.shape[0]

    W = np.concatenate([W_pos, W_feat], axis=1)  # [384, 95]
    WT = np.ascontiguousarray(W.T).astype(ml_dtypes.bfloat16)  # [95, 384]

    order = np.argsort(ids, kind="stable")
    counts = np.bincount(ids, minlength=NCL)
    # Ks[c] = ceil(ceil(n_c / 8) / 512) supertiles per cluster per core
    Ks = [
        ((int(counts[c]) + N_CORES - 1) // N_CORES + ST - 1) // ST
        for c in range(NCL)
    ]
    schedule = [c for c in range(NCL) for _ in range(Ks[c])]
    while len(schedule) % (BANK * OUTG):
        schedule.append(0)
    T = len(schedule)
    R = T * ST
    NB = T // BANK

    offs = np.concatenate([[0], np.cumsum(counts)])
    # Per-core row lists: cluster c's shard for core m is Ic[m::8], padded to
    # Ks[c]*512 with index N (an all-zero row appended to X).
    rows = np.full((N_CORES, R), N, dtype=np.int64)
    tile_base = 0
    for c in range(NCL):
        Ic = order[offs[c] : offs[c + 1]]
        for m in range(N_CORES):
            sh = Ic[m::N_CORES]
            rows[m, tile_base * ST : tile_base * ST + len(sh)] = sh
        tile_base += Ks[c]

    Xaug = np.zeros((N + 1, F), dtype=ml_dtypes.bfloat16)
    Xaug[:N] = X

    in_maps = []
    for m in range(N_CORES):
        Xt = np.ascontiguousarray(Xaug[rows[m]].T)  # [95, R] bf16
        in_maps.append({"xt": Xt, "wt": WT})

    key = (tuple(schedule), R)
    if key not in _prog_cache:
        _prog_cache.clear()
        _prog_cache[key] = _build_program(schedule, R)
    nc = _prog_cache[key]

    from concourse.bass_utils import run_bass_kernel_spmd

    res = run_bass_kernel_spmd(nc, in_maps, list(range(N_CORES)))

    out = np.zeros((N, 3), dtype=np.float32)
    for m in range(N_CORES):
        otm = np.asarray(res.results[m]["ot"])  # [12, NB*512] bf16
        # ot[3q+r, 512b+m] = out channel r of supertile (b*BANK+q) row m
        full = (
            otm.reshape(BANK, 3, NB, ST)
            .transpose(2, 0, 3, 1)
            .reshape(R, 3)
            .astype(np.float32)
        )
        valid = rows[m] != N
        out[rows[m][valid]] = full[valid]
    return out
